# revision 1
# baseline (speedup 1.0000x reference)
"""MeshUnpool Trainium2 kernel.

For every fine edge slot s in [0, 16384):
  - if s is a kept slot (s == keep_idx[j] for some j): out[s] = x_coarse[j]
  - else: out[s] = x_coarse[argmin_j |keep_idx[j] - s|]  (first-min tie-break)

Every output row is a gathered x_coarse row; the device computes the gather
index per slot with an O(E_fine) scan algorithm instead of the naive
(16384 x 8192) distance matrix:

  1. build the slot table with a matmul scatter: one-hot matrices
     A[j, p] = (keep_j >> 7 == p) and C[j, f] = (keep_j & 127 == f) are
     built with two wide compares, then T[p, f] = sum_j A[j,p]*C[j,f]*v_j
     accumulates on the TensorEngine (v = j_hi+1 and j_lo payloads).
     T is the scatter: slot s = 128p+f holds its keep's j, or 0 if missing.
  2. prefix-max scan of key1/key2 over slots -> nearest kept slot <= s with
     its j riding along; suffix-min scan -> nearest kept slot >= s.
     Scans run per-partition with tensor_tensor_scan plus a transposed
     cross-partition carry fixup.
  3. elementwise distance compare + first-min tie-break -> src_idx per slot.
  4. a one-hot matmul extracts this core's 2048 indices, then 16 indirect
     row-gather DMAs (128 rows x 2 KB) pull the output rows from x_coarse.

Work is sharded over 8 cores by rows of the fine-edge dim; x_coarse and
keep_idx are replicated; each core fills its 2048-row slice.
"""

import os
import sys

import numpy as np

E_FINE = 16384
E_COARSE = 8192
C = 512
N_CORES = 8
SLICE = E_FINE // N_CORES  # 2048
P = 128
NBLK = SLICE // P  # 16 gather blocks per core
KC = E_COARSE // P  # 64 keep columns (j = 64*p + c)

KEY_OFF = 2097152.0  # +2^21 added to keys so "missing slot" == 0.0
R_SENT = 8388608.0   # +2^23: flipped sentinel for the suffix-min scans

_NC_CACHE = {}
_DUMP = None  # debug hook: _DUMP(name, ap) dumps an AP to a DRAM tensor


def _dump(name, ap):
    if _DUMP is not None:
        _DUMP(name, ap)


def _ensure_paths():
    for p in ("/opt/trn_rl_repo", "/root/.axon_site/_ro/trn_rl_repo"):
        if os.path.isdir(p) and p not in sys.path:
            sys.path.append(p)


def build_program(nc, bass, mybir, tile):
    f32 = mybir.dt.float32
    i32 = mybir.dt.int32
    Alu = mybir.AluOpType

    i16 = mybir.dt.int16

    bf16 = mybir.dt.bfloat16

    xc = nc.dram_tensor("xc", [E_COARSE, C], f32, kind="ExternalInput")
    # keep_w[jp, c] = keep_idx[c*128 + jp]  (j on partitions per chunk c)
    keep_w = nc.dram_tensor("keep_w", [P, KC], i32, kind="ExternalInput")
    # jhi1[jp, c] = (j >> 6) + 1 and jlo[jp, 0] = j & 63  for j = c*128+jp
    jhi1_in = nc.dram_tensor("jhi1", [P, KC], bf16, kind="ExternalInput")
    jlo_in = nc.dram_tensor("jlo", [P, 1], f32, kind="ExternalInput")
    # iota_b[p, f] = f  (same row on every partition)
    iota_in = nc.dram_tensor("iotab", [P, P], bf16, kind="ExternalInput")
    ident = nc.dram_tensor("ident", [P, P], f32, kind="ExternalInput")
    sel = nc.dram_tensor("sel", [P, NBLK], f32, kind="ExternalInput")
    y = nc.dram_tensor("y", [SLICE, C], f32, kind="ExternalOutput")

    with tile.TileContext(nc) as tc:
        with (
            tc.tile_pool(name="sb", bufs=1) as sb,
            tc.tile_pool(name="ps", bufs=1, space="PSUM") as ps,
            tc.tile_pool(name="gp", bufs=8) as gp,
        ):
            keep_t = sb.tile([P, KC], i32)
            nc.sync.dma_start(keep_t[:], keep_w[:])
            jhi1_t = sb.tile([P, KC], bf16)
            nc.sync.dma_start(jhi1_t[:], jhi1_in[:])
            jlo_t = sb.tile([P, 1], f32)
            nc.sync.dma_start(jlo_t[:], jlo_in[:])
            iota_t = sb.tile([P, P], bf16)
            nc.sync.dma_start(iota_t[:], iota_in[:])
            ident_t = sb.tile([P, P], f32)
            nc.sync.dma_start(ident_t[:], ident[:])
            sel_t = sb.tile([P, NBLK], f32)
            nc.sync.dma_start(sel_t[:], sel[:])

            # slot position iota: pos[p, f] = 16384 + 128p + f (the 16384
            # base makes the +2^21 key offset fall out of 128*pos)
            pos = sb.tile([P, P], i32)
            nc.gpsimd.iota(
                pos[:], pattern=[[1, P]], base=16384, channel_multiplier=P
            )

            # matmul scatter: T[p, f] = sum_j [keep_j>>7 == p][keep_j&127 == f] * v_j
            hi_i = sb.tile([P, KC], i32)
            nc.vector.tensor_scalar(hi_i[:], keep_t[:], 7, None, Alu.arith_shift_right)
            lo_i = sb.tile([P, KC], i32)
            nc.vector.tensor_scalar(lo_i[:], keep_t[:], 127, None, Alu.bitwise_and)
            hi_b = sb.tile([P, KC], bf16)
            nc.vector.tensor_copy(hi_b[:], hi_i[:])
            lo_b = sb.tile([P, KC], bf16)
            nc.vector.tensor_copy(lo_b[:], lo_i[:])

            # split builds into halves so the PE can start on half 0 while
            # the vector engine builds half 1; A on gpsimd runs in parallel
            # with C/Ch on vector, Cl on the scalar engine
            a_all = sb.tile([P, KC, P], bf16)
            cmat = sb.tile([P, KC, P], bf16)
            chmat = sb.tile([P, KC, P], bf16)
            clmat = sb.tile([P, KC, P], bf16)
            HC = KC // 2
            for h in range(2):
                cs = slice(h * HC, (h + 1) * HC)
                nc.vector.tensor_tensor(
                    a_all[:, cs, :],
                    hi_b[:, cs].unsqueeze(2).to_broadcast([P, HC, P]),
                    iota_t[:].unsqueeze(1).to_broadcast([P, HC, P]),
                    Alu.is_equal,
                )
                nc.vector.tensor_tensor(
                    cmat[:, cs, :],
                    lo_b[:, cs].unsqueeze(2).to_broadcast([P, HC, P]),
                    iota_t[:].unsqueeze(1).to_broadcast([P, HC, P]),
                    Alu.is_equal,
                )
                nc.vector.tensor_tensor(
                    chmat[:, cs, :],
                    cmat[:, cs, :],
                    jhi1_t[:, cs].unsqueeze(2).to_broadcast([P, HC, P]),
                    Alu.mult,
                )
                nc.scalar.mul(clmat[:, cs, :], cmat[:, cs, :], jlo_t[:, 0:1])

            tph = ps.tile([P, P], f32)
            tpl = ps.tile([P, P], f32)
            for c in range(KC):
                nc.tensor.matmul(
                    tph[:],
                    a_all[:, c, :],
                    chmat[:, c, :],
                    start=(c == 0),
                    stop=(c == KC - 1),
                )
                nc.tensor.matmul(
                    tpl[:],
                    a_all[:, c, :],
                    clmat[:, c, :],
                    start=(c == 0),
                    stop=(c == KC - 1),
                )
            posf = sb.tile([P, P], f32)
            nc.vector.tensor_copy(posf[:], pos[:])
            m_kept = sb.tile([P, P], f32)
            nc.vector.tensor_scalar(m_kept[:], tph[:], 0.0, None, Alu.is_gt)
            th = sb.tile([P, P], f32)
            nc.vector.tensor_scalar(th[:], tph[:], 1.0, None, Alu.subtract)
            # key1 = kept * (128*pos + j_hi); 128*pos = 128*slot + 2^21
            k1r = sb.tile([P, P], f32)
            nc.vector.scalar_tensor_tensor(
                k1r[:], posf[:], 128.0, th[:], Alu.mult, Alu.add
            )
            key1 = sb.tile([P, P], f32)
            nc.vector.tensor_tensor(key1[:], k1r[:], m_kept[:], Alu.mult)
            # key2 = kept * (64*pos + j_lo); 64*pos = 64*slot + 2^20
            k2r = sb.tile([P, P], f32)
            nc.vector.scalar_tensor_tensor(
                k2r[:], posf[:], 64.0, tpl[:], Alu.mult, Alu.add
            )
            key2 = sb.tile([P, P], f32)
            nc.vector.tensor_tensor(key2[:], k2r[:], m_kept[:], Alu.mult)
            _dump("d_key1", key1[:])
            _dump("d_key2", key2[:])

            # right-scan keys: missing slots (0.0) flipped to +R_SENT
            msk = sb.tile([P, P], f32)
            nc.vector.tensor_scalar(msk[:], key1[:], 0.0, None, Alu.is_equal)
            r1 = sb.tile([P, P], f32)
            nc.vector.scalar_tensor_tensor(
                r1[:], msk[:], R_SENT, key1[:], Alu.mult, Alu.add
            )
            r2 = sb.tile([P, P], f32)
            nc.vector.scalar_tensor_tensor(
                r2[:], msk[:], R_SENT, key2[:], Alu.mult, Alu.add
            )

            # per-partition scans (free axis); suffix scans via reversed APs
            l1s = sb.tile([P, P], f32)
            nc.vector.tensor_tensor_scan(
                l1s[:], key1[:], key1[:], 0.0, Alu.max, Alu.max
            )
            l2s = sb.tile([P, P], f32)
            nc.vector.tensor_tensor_scan(
                l2s[:], key2[:], key2[:], 0.0, Alu.max, Alu.max
            )
            r1s = sb.tile([P, P], f32)
            nc.vector.tensor_tensor_scan(
                r1s[:, P - 1 :: -1],
                r1[:, P - 1 :: -1],
                r1[:, P - 1 :: -1],
                R_SENT,
                Alu.min,
                Alu.min,
            )
            r2s = sb.tile([P, P], f32)
            nc.vector.tensor_tensor_scan(
                r2s[:, P - 1 :: -1],
                r2[:, P - 1 :: -1],
                r2[:, P - 1 :: -1],
                R_SENT,
                Alu.min,
                Alu.min,
            )

            # cross-partition carry: transpose per-partition totals, exclusive
            # scan along the row, transpose back, combine
            totL = sb.tile([P, 2], f32)
            nc.vector.tensor_copy(totL[:, 0:1], l1s[:, P - 1 : P])
            nc.vector.tensor_copy(totL[:, 1:2], l2s[:, P - 1 : P])
            totR = sb.tile([P, 2], f32)
            nc.vector.tensor_copy(totR[:, 0:1], r1s[:, 0:1])
            nc.vector.tensor_copy(totR[:, 1:2], r2s[:, 0:1])
            totL_tp = ps.tile([2, P], f32)
            nc.tensor.transpose(totL_tp[:], totL[:], ident_t[:])
            totL_T = sb.tile([2, P], f32)
            nc.vector.tensor_copy(totL_T[:], totL_tp[:])
            totR_tp = ps.tile([2, P], f32)
            nc.tensor.transpose(totR_tp[:], totR[:], ident_t[:])
            totR_T = sb.tile([2, P], f32)
            nc.vector.tensor_copy(totR_T[:], totR_tp[:])

            exL = sb.tile([2, P], f32)
            nc.vector.memset(exL[:, 0:1], 0.0)
            nc.vector.tensor_tensor_scan(
                exL[:, 1:P],
                totL_T[:, 0 : P - 1],
                totL_T[:, 0 : P - 1],
                0.0,
                Alu.max,
                Alu.max,
            )
            exR = sb.tile([2, P], f32)
            nc.vector.memset(exR[:, P - 1 : P], R_SENT)
            nc.vector.tensor_tensor_scan(
                exR[:, P - 2 :: -1],
                totR_T[:, P - 1 : 0 : -1],
                totR_T[:, P - 1 : 0 : -1],
                R_SENT,
                Alu.min,
                Alu.min,
            )
            exL_tp = ps.tile([P, 2], f32)
            nc.tensor.transpose(exL_tp[:], exL[:], ident_t[0:2, 0:2])
            carryL = sb.tile([P, 2], f32)
            nc.vector.tensor_copy(carryL[:], exL_tp[:])
            exR_tp = ps.tile([P, 2], f32)
            nc.tensor.transpose(exR_tp[:], exR[:], ident_t[0:2, 0:2])
            carryR = sb.tile([P, 2], f32)
            nc.vector.tensor_copy(carryR[:], exR_tp[:])
            nc.vector.tensor_scalar_max(l1s[:], l1s[:], carryL[:, 0:1])
            nc.vector.tensor_scalar_max(l2s[:], l2s[:], carryL[:, 1:2])
            nc.vector.tensor_scalar_min(r1s[:], r1s[:], carryR[:, 0:1])
            nc.vector.tensor_scalar_min(r2s[:], r2s[:], carryR[:, 1:2])
            _dump("d_l1s", l1s[:])
            _dump("d_l2s", l2s[:])
            _dump("d_r1s", r1s[:])
            _dump("d_r2s", r2s[:])

            # decode: slot = key1>>7, j = ((key1&127)<<6) | (key2&63)
            l1i = sb.tile([P, P], i32)
            nc.vector.tensor_copy(l1i[:], l1s[:])
            l2i = sb.tile([P, P], i32)
            nc.vector.tensor_copy(l2i[:], l2s[:])
            r1i = sb.tile([P, P], i32)
            nc.vector.tensor_copy(r1i[:], r1s[:])
            r2i = sb.tile([P, P], i32)
            nc.vector.tensor_copy(r2i[:], r2s[:])

            slot_l = sb.tile([P, P], i32)
            nc.vector.tensor_scalar(slot_l[:], l1i[:], 7, None, Alu.arith_shift_right)
            slot_r = sb.tile([P, P], i32)
            nc.vector.tensor_scalar(slot_r[:], r1i[:], 7, None, Alu.arith_shift_right)
            jhl = sb.tile([P, P], i32)
            nc.vector.tensor_scalar(
                jhl[:], l1i[:], 127, 6, Alu.bitwise_and, Alu.arith_shift_left
            )
            jll = sb.tile([P, P], i32)
            nc.vector.tensor_scalar(jll[:], l2i[:], 63, None, Alu.bitwise_and)
            jl = sb.tile([P, P], i32)
            nc.vector.tensor_tensor(jl[:], jhl[:], jll[:], Alu.bitwise_or)
            jhr = sb.tile([P, P], i32)
            nc.vector.tensor_scalar(
                jhr[:], r1i[:], 127, 6, Alu.bitwise_and, Alu.arith_shift_left
            )
            jlr = sb.tile([P, P], i32)
            nc.vector.tensor_scalar(jlr[:], r2i[:], 63, None, Alu.bitwise_and)
            jr = sb.tile([P, P], i32)
            nc.vector.tensor_tensor(jr[:], jhr[:], jlr[:], Alu.bitwise_or)

            dl = sb.tile([P, P], i32)
            nc.vector.tensor_tensor(dl[:], pos[:], slot_l[:], Alu.subtract)
            drr = sb.tile([P, P], i32)
            nc.vector.tensor_tensor(drr[:], slot_r[:], pos[:], Alu.subtract)
            m_l = sb.tile([P, P], i32)
            nc.vector.tensor_tensor(m_l[:], dl[:], drr[:], Alu.is_lt)
            m_r = sb.tile([P, P], i32)
            nc.vector.tensor_tensor(m_r[:], drr[:], dl[:], Alu.is_lt)
            src = sb.tile([P, P], i32)
            nc.vector.tensor_tensor(src[:], jl[:], jr[:], Alu.min)
            nc.vector.copy_predicated(src[:], m_r[:], jr[:])
            nc.vector.copy_predicated(src[:], m_l[:], jl[:])
            _dump("d_src", src[:])
            _dump("d_pos", pos[:])
            srcf = sb.tile([P, P], f32)
            nc.vector.tensor_copy(srcf[:], src[:])

            # extract this core's 16 blocks of 128 indices: G[r, g] =
            # src[16m+g, r] via one-hot matmul, then gather + write out
            g_ps = ps.tile([P, NBLK], f32)
            nc.tensor.matmul(g_ps[:], srcf[:], sel_t[:], start=True, stop=True)
            g_i = sb.tile([P, NBLK], i32)
            nc.vector.tensor_copy(g_i[:], g_ps[:])
            _dump("d_gi", g_i[:])

            for b in range(NBLK):
                gt = gp.tile([P, C], f32, tag="g")
                nc.gpsimd.indirect_dma_start(
                    out=gt[:],
                    out_offset=None,
                    in_=xc[:],
                    in_offset=bass.IndirectOffsetOnAxis(
                        ap=g_i[:, b : b + 1], axis=0
                    ),
                )
                nc.sync.dma_start(y[b * P : (b + 1) * P, :], gt[:])

    return {"y": y}


def host_inputs(x_coarse, keep_idx):
    import ml_dtypes

    bf = ml_dtypes.bfloat16
    x_coarse = np.ascontiguousarray(np.asarray(x_coarse), dtype=np.float32)
    ki = np.ascontiguousarray(np.asarray(keep_idx), dtype=np.int32).reshape(-1)
    # j = c*128 + jp: keep_w[jp, c] = keep_idx[j]
    keep_w = np.ascontiguousarray(ki.reshape(KC, P).T)
    pp = np.arange(P)
    cc = np.arange(KC)
    jhi1_a = (2 * cc[None, :] + (pp[:, None] >= 64) + 1).astype(bf)
    jlo_a = (pp[:, None] & 63).astype(np.float32)
    iota_a = np.tile(np.arange(P), (P, 1)).astype(bf)
    ident_a = np.eye(P, dtype=np.float32)
    base = {
        "xc": x_coarse,
        "keep_w": keep_w,
        "jhi1": np.ascontiguousarray(jhi1_a),
        "jlo": np.ascontiguousarray(jlo_a),
        "iotab": np.ascontiguousarray(iota_a),
        "ident": ident_a,
    }
    in_maps = []
    for m in range(N_CORES):
        sel_a = np.zeros((P, NBLK), dtype=np.float32)
        sel_a[16 * m + np.arange(NBLK), np.arange(NBLK)] = 1.0
        in_maps.append(dict(base, sel=sel_a))
    return in_maps


def _get_nc():
    if "nc" in _NC_CACHE:
        return _NC_CACHE["nc"]
    _ensure_paths()
    from concourse import bass, mybir
    import concourse.bacc as bacc
    import concourse.tile as tile

    nc = bacc.Bacc("TRN2", target_bir_lowering=False, debug=False, dynamic_dma_scratch_size=16384)
    build_program(nc, bass, mybir, tile)
    nc.compile()
    _NC_CACHE["nc"] = nc
    return nc


def run_on_hw(in_maps, trace=False, **kwargs):
    _ensure_paths()
    from concourse.bass_utils import run_bass_kernel_spmd

    nc = _get_nc()
    return run_bass_kernel_spmd(
        nc, in_maps, core_ids=list(range(N_CORES)), trace=trace, **kwargs
    )


def kernel(x_coarse, keep_idx, E_fine=None, **_unused):
    in_maps = host_inputs(x_coarse, keep_idx)
    res = run_on_hw(in_maps)
    out = np.concatenate([res.results[m]["y"] for m in range(N_CORES)], axis=0)
    return np.ascontiguousarray(out.astype(np.float32, copy=False))



# revision 10
# speedup vs baseline: 1.0757x; 1.0757x over previous
"""MeshUnpool Trainium2 kernel (v3).

For every fine edge slot s in [0, 16384):
  - if s is a kept slot (s == keep_idx[j] for some j): out[s] = x_coarse[j]
  - else: out[s] = x_coarse[argmin_j |keep_idx[j] - s|]  (first-min tie-break)

Algorithm (per core; front-end replicated, each core extracts its slice):
  1. slot table via matmul scatter in chunk-major layout (PE operands
     contiguous). Payload u = jp+1 rides a 4x tensor_scalar; payload v = c
     rides a host-constant c-ramp multiply (2x). One-hot compares split
     between vector and gpsimd engines.
  2. dual-f32-key prefix-max / suffix-min scans give nearest kept slot
     left/right with (u, v) riding along; cross-partition carry via PE
     transpose + exclusive scan.
  3. integer decode + first-min tie-break -> src row per slot.
  4. one sel-matmul extracts this core's 2048 indices; 4 multi-offset
     indirect DMAs (512 rows each) gather x_coarse rows; 4 strided DMAs
     write the slice of y.

Sharding: rows of the fine-edge dim, 2048 per core; x_coarse/keep replicated.
"""

import os
import sys

import numpy as np

E_FINE = 16384
E_COARSE = 8192
C = 512
N_CORES = 8
SLICE = E_FINE // N_CORES  # 2048
P = 128
KC = E_COARSE // P  # 64 keep chunks (j = 128*c + jp)
NPC = 4  # build pieces
PCK = KC // NPC  # chunks per piece
NG = 4  # gather/write groups
GB = 4  # blocks of 128 rows per group

R_SENT = 8388608.0  # 2^23 sentinel for the suffix-min scans

A_ALL_ON_GPSIMD = False  # risk flag: one-hot A build on Pool engine
MULTI_COL_INDIRECT = False  # risk flag: [128, 4] offset APs per indirect DMA

_NC_CACHE = {}
_DUMP = None  # debug hook: _DUMP(name, ap) dumps an AP to a DRAM tensor


def _dump(name, ap):
    if _DUMP is not None:
        _DUMP(name, ap)


def _ensure_paths():
    for p in ("/opt/trn_rl_repo", "/root/.axon_site/_ro/trn_rl_repo"):
        if os.path.isdir(p) and p not in sys.path:
            sys.path.append(p)


def build_program(nc, bass, mybir, tile):
    f32 = mybir.dt.float32
    i32 = mybir.dt.int32
    bf16 = mybir.dt.bfloat16
    Alu = mybir.AluOpType

    xc = nc.dram_tensor("xc", [E_COARSE, C], f32, kind="ExternalInput")
    # bf16 pack: hb | lb | iota  (hb/lb[jp, c] = keep_idx[128*c+jp] >>7 / &127)
    bpk = nc.dram_tensor("bpk", [P, 2 * KC + P], bf16, kind="ExternalInput")
    # crp[jp, cc, t] = global chunk index (c-ramp payload), one per piece
    crps = [
        nc.dram_tensor(f"crp{k}", [P, PCK, P], bf16, kind="ExternalInput")
        for k in range(NPC)
    ]
    # f32 pack: pos128m1 | pos64 | jp1 | sel | ident
    fpk = nc.dram_tensor("fpk", [P, 401], f32, kind="ExternalInput")
    # i32 pack: pos
    ipk = nc.dram_tensor("ipk", [P, P], i32, kind="ExternalInput")
    y = nc.dram_tensor("y", [SLICE, C], f32, kind="ExternalOutput")

    with tile.TileContext(nc) as tc:
        with (
            tc.tile_pool(name="sb", bufs=1) as sb,
            tc.tile_pool(name="ps", bufs=1, space="PSUM") as ps,
        ):
            # ---- input loads (spread over both HWDGE engines)
            bpk_t = sb.tile([P, 2 * KC + P], bf16)
            nc.sync.dma_start(bpk_t[:], bpk[:])
            fpk_t = sb.tile([P, 401], f32)
            nc.scalar.dma_start(fpk_t[:], fpk[:])
            pos_i = sb.tile([P, P], i32)
            nc.scalar.dma_start(pos_i[:], ipk[:])

            hb = bpk_t[:, 0:KC]
            lb = bpk_t[:, KC : 2 * KC]
            iota = bpk_t[:, 2 * KC : 2 * KC + P]
            pos128m1 = fpk_t[:, 0:128]
            pos64 = fpk_t[:, 128:256]
            jp1 = fpk_t[:, 256:257]
            sel = fpk_t[:, 257:273]
            ident = fpk_t[:, 273:401]

            tps = ps.tile([P, 2 * P], f32)
            a_eng = nc.gpsimd if A_ALL_ON_GPSIMD else nc.vector

            # ---- one-hot builds + matmul scatter, piecewise over chunks
            for k in range(NPC):
                cs = slice(k * PCK, (k + 1) * PCK)
                crp_t = sb.tile([P, PCK, P], bf16, tag=f"crp{k}")
                nc.scalar.dma_start(crp_t[:], crps[k][:])
                # aA[jp, cc, t] = (hb[jp, c] == t)
                aA = sb.tile([P, PCK, P], bf16, tag=f"aA{k}")
                a_eng.tensor_tensor(
                    aA[:],
                    hb[:, cs].unsqueeze(2).to_broadcast([P, PCK, P]),
                    iota.unsqueeze(1).to_broadcast([P, PCK, P]),
                    Alu.is_equal,
                )
                # cT[jp, cc, t] = (lb[jp, c] == t)
                cT = sb.tile([P, PCK, P], bf16, tag=f"cT{k}")
                nc.vector.tensor_tensor(
                    cT[:],
                    lb[:, cs].unsqueeze(2).to_broadcast([P, PCK, P]),
                    iota.unsqueeze(1).to_broadcast([P, PCK, P]),
                    Alu.is_equal,
                )
                # pay_u = (jp+1) * cT ; pay_v = c * cT
                pay = sb.tile([P, PCK, 2 * P], bf16, tag=f"pay{k}")
                nc.vector.tensor_scalar(
                    pay[:, :, 0:P], cT[:], jp1, None, Alu.mult
                )
                nc.vector.tensor_tensor(
                    pay[:, :, P : 2 * P], cT[:], crp_t[:], Alu.mult
                )
                for cc in range(PCK):
                    c = k * PCK + cc
                    nc.tensor.matmul(
                        tps[:],
                        aA[:, cc, :],
                        pay[:, cc, :],
                        start=(c == 0),
                        stop=(c == KC - 1),
                    )

            # ---- keys: key1 = kept*(128*pos + jp), key2 = kept*(64*pos + c)
            mk = sb.tile([P, P], f32)
            nc.vector.tensor_scalar(mk[:], tps[:, 0:P], 0.0, None, Alu.is_gt)
            k1r = sb.tile([P, P], f32)
            nc.vector.tensor_tensor(k1r[:], tps[:, 0:P], pos128m1, Alu.add)
            key1 = sb.tile([P, P], f32)
            nc.vector.tensor_tensor(key1[:], k1r[:], mk[:], Alu.mult)
            k2r = sb.tile([P, P], f32)
            nc.vector.tensor_tensor(k2r[:], tps[:, P : 2 * P], pos64, Alu.add)
            key2 = sb.tile([P, P], f32)
            nc.vector.tensor_tensor(key2[:], k2r[:], mk[:], Alu.mult)
            msk = sb.tile([P, P], f32)
            nc.vector.tensor_scalar(msk[:], key1[:], 0.0, None, Alu.is_equal)
            r1 = sb.tile([P, P], f32)
            nc.vector.scalar_tensor_tensor(
                r1[:], msk[:], R_SENT, key1[:], Alu.mult, Alu.add
            )
            r2 = sb.tile([P, P], f32)
            nc.vector.scalar_tensor_tensor(
                r2[:], msk[:], R_SENT, key2[:], Alu.mult, Alu.add
            )
            _dump("d_key1", key1[:])
            _dump("d_key2", key2[:])

            # ---- per-partition scans; l1s|l2s|r1s|r2s packed in one tile
            sc = sb.tile([P, 4 * P], f32)
            nc.vector.tensor_tensor_scan(
                sc[:, 0:P], key1[:], key1[:], 0.0, Alu.max, Alu.max
            )
            nc.vector.tensor_tensor_scan(
                sc[:, P : 2 * P], key2[:], key2[:], 0.0, Alu.max, Alu.max
            )
            nc.vector.tensor_tensor_scan(
                sc[:, 3 * P - 1 : 2 * P - 1 : -1],
                r1[:, P - 1 :: -1],
                r1[:, P - 1 :: -1],
                R_SENT,
                Alu.min,
                Alu.min,
            )
            nc.vector.tensor_tensor_scan(
                sc[:, 4 * P - 1 : 3 * P - 1 : -1],
                r2[:, P - 1 :: -1],
                r2[:, P - 1 :: -1],
                R_SENT,
                Alu.min,
                Alu.min,
            )

            # ---- cross-partition carries: transpose totals, exclusive scan
            totL_ps = ps.tile([2, P], f32)
            nc.tensor.transpose(totL_ps[:], sc[:, P - 1 : 2 * P : P], ident)
            totR_ps = ps.tile([2, P], f32)
            nc.tensor.transpose(totR_ps[:], sc[:, 2 * P : 3 * P + 1 : P], ident)
            totL_T = sb.tile([2, P], f32)
            nc.vector.tensor_copy(totL_T[:], totL_ps[:])
            totR_T = sb.tile([2, P], f32)
            nc.vector.tensor_copy(totR_T[:], totR_ps[:])
            exL = sb.tile([2, P], f32)
            nc.vector.memset(exL[:, 0:1], 0.0)
            nc.vector.tensor_tensor_scan(
                exL[:, 1:P],
                totL_T[:, 0 : P - 1],
                totL_T[:, 0 : P - 1],
                0.0,
                Alu.max,
                Alu.max,
            )
            exR = sb.tile([2, P], f32)
            nc.vector.memset(exR[:, P - 1 : P], R_SENT)
            nc.vector.tensor_tensor_scan(
                exR[:, P - 2 :: -1],
                totR_T[:, P - 1 : 0 : -1],
                totR_T[:, P - 1 : 0 : -1],
                R_SENT,
                Alu.min,
                Alu.min,
            )
            carL_ps = ps.tile([P, 2], f32)
            nc.tensor.transpose(carL_ps[:], exL[:], ident[0:2, 0:2])
            carR_ps = ps.tile([P, 2], f32)
            nc.tensor.transpose(carR_ps[:], exR[:], ident[0:2, 0:2])
            carL = sb.tile([P, 2], f32)
            nc.vector.tensor_copy(carL[:], carL_ps[:])
            carR = sb.tile([P, 2], f32)
            nc.vector.tensor_copy(carR[:], carR_ps[:])

            # ---- combine carries (f32), then integer decode
            nc.vector.tensor_scalar_max(sc[:, 0:P], sc[:, 0:P], carL[:, 0:1])
            nc.vector.tensor_scalar_max(
                sc[:, P : 2 * P], sc[:, P : 2 * P], carL[:, 1:2]
            )
            nc.vector.tensor_scalar_min(
                sc[:, 2 * P : 3 * P], sc[:, 2 * P : 3 * P], carR[:, 0:1]
            )
            nc.vector.tensor_scalar_min(
                sc[:, 3 * P : 4 * P], sc[:, 3 * P : 4 * P], carR[:, 1:2]
            )
            l1m = sb.tile([P, P], i32)
            nc.vector.tensor_copy(l1m[:], sc[:, 0:P])
            l2m = sb.tile([P, P], i32)
            nc.vector.tensor_copy(l2m[:], sc[:, P : 2 * P])
            r1m = sb.tile([P, P], i32)
            nc.vector.tensor_copy(r1m[:], sc[:, 2 * P : 3 * P])
            r2m = sb.tile([P, P], i32)
            nc.vector.tensor_copy(r2m[:], sc[:, 3 * P : 4 * P])
            _dump("d_l1s", l1m[:])
            _dump("d_r1s", r1m[:])

            slot_l = sb.tile([P, P], i32)
            nc.vector.tensor_scalar(slot_l[:], l1m[:], 7, None, Alu.arith_shift_right)
            slot_r = sb.tile([P, P], i32)
            nc.vector.tensor_scalar(slot_r[:], r1m[:], 7, None, Alu.arith_shift_right)
            # j = 128*c + jp  (u = jp+1 was the payload, key stores jp; v = c)
            jcl = sb.tile([P, P], i32)
            nc.vector.tensor_scalar(
                jcl[:], l2m[:], 63, 7, Alu.bitwise_and, Alu.arith_shift_left
            )
            jpl = sb.tile([P, P], i32)
            nc.vector.tensor_scalar(jpl[:], l1m[:], 127, None, Alu.bitwise_and)
            jl = sb.tile([P, P], i32)
            nc.vector.tensor_tensor(jl[:], jcl[:], jpl[:], Alu.bitwise_or)
            jcr = sb.tile([P, P], i32)
            nc.vector.tensor_scalar(
                jcr[:], r2m[:], 63, 7, Alu.bitwise_and, Alu.arith_shift_left
            )
            jpr = sb.tile([P, P], i32)
            nc.vector.tensor_scalar(jpr[:], r1m[:], 127, None, Alu.bitwise_and)
            jr = sb.tile([P, P], i32)
            nc.vector.tensor_tensor(jr[:], jcr[:], jpr[:], Alu.bitwise_or)

            dl = sb.tile([P, P], i32)
            nc.vector.tensor_tensor(dl[:], pos_i[:], slot_l[:], Alu.subtract)
            drr = sb.tile([P, P], i32)
            nc.vector.tensor_tensor(drr[:], slot_r[:], pos_i[:], Alu.subtract)
            ml = sb.tile([P, P], i32)
            nc.vector.tensor_tensor(ml[:], dl[:], drr[:], Alu.is_lt)
            mr = sb.tile([P, P], i32)
            nc.vector.tensor_tensor(mr[:], drr[:], dl[:], Alu.is_lt)
            src = sb.tile([P, P], i32)
            nc.vector.tensor_tensor(src[:], jl[:], jr[:], Alu.min)
            nc.vector.copy_predicated(src[:], mr[:], jr[:])
            nc.vector.copy_predicated(src[:], ml[:], jl[:])
            _dump("d_src", src[:])
            srcf = sb.tile([P, P], f32)
            nc.vector.tensor_copy(srcf[:], src[:])

            # ---- extract this core's 2048 indices: g_i[p, b] = src row for
            # slot 2048m + 128b + p
            g_ps = ps.tile([P, 16], f32)
            nc.tensor.matmul(g_ps[:], srcf[:], sel, start=True, stop=True)
            g_i = sb.tile([P, 16], i32)
            nc.vector.tensor_copy(g_i[:], g_ps[:])
            _dump("d_gi", g_i[:])

            # ---- gather + write: per-block indirect gathers (v1 pattern)
            with tc.tile_pool(name="gp", bufs=8) as gp:
                for b in range(16):
                    gt = gp.tile([P, C], f32, tag="g")
                    nc.gpsimd.indirect_dma_start(
                        out=gt[:],
                        out_offset=None,
                        in_=xc[:],
                        in_offset=bass.IndirectOffsetOnAxis(
                            ap=g_i[:, b : b + 1], axis=0
                        ),
                    )
                    nc.sync.dma_start(y[b * P : (b + 1) * P, :], gt[:])

    return {"y": y}


def host_inputs(x_coarse, keep_idx):
    import ml_dtypes

    bf = ml_dtypes.bfloat16
    x_coarse = np.ascontiguousarray(np.asarray(x_coarse), dtype=np.float32)
    ki = np.ascontiguousarray(np.asarray(keep_idx), dtype=np.int32).reshape(-1)
    # j = 128*c + jp: keep_w[jp, c] = keep_idx[j]
    keep_w = ki.reshape(KC, P).T
    hb_a = (keep_w >> 7).astype(bf)
    lb_a = (keep_w & 127).astype(bf)

    t = np.arange(P)
    iota_a = np.tile(t[None, :], (P, 1)).astype(bf)
    crp_full = np.tile(np.arange(KC)[None, :, None], (P, 1, P)).astype(bf)

    pos = (16384 + 128 * t[:, None] + t[None, :]).astype(np.int64)
    pos128m1 = (128 * pos - 1).astype(np.float32)
    pos64 = (64 * pos).astype(np.float32)
    jp1 = (t[:, None] + 1).astype(np.float32)
    ident_a = np.eye(P, dtype=np.float32)

    base = {
        "xc": x_coarse,
        "bpk": np.ascontiguousarray(np.concatenate([hb_a, lb_a, iota_a], axis=1)),
        "ipk": np.ascontiguousarray(pos.astype(np.int32)),
    }
    for k in range(NPC):
        base[f"crp{k}"] = np.ascontiguousarray(
            crp_full[:, k * PCK : (k + 1) * PCK, :]
        )

    in_maps = []
    for m in range(N_CORES):
        sel_a = np.zeros((P, 16), dtype=np.float32)
        sel_a[16 * m + np.arange(16), np.arange(16)] = 1.0
        fpk_a = np.concatenate([pos128m1, pos64, jp1, sel_a, ident_a], axis=1)
        in_maps.append(dict(base, fpk=np.ascontiguousarray(fpk_a)))
    return in_maps


def _get_nc():
    if "nc" in _NC_CACHE:
        return _NC_CACHE["nc"]
    _ensure_paths()
    from concourse import bass, mybir
    import concourse.bacc as bacc
    import concourse.tile as tile

    nc = bacc.Bacc(
        "TRN2", target_bir_lowering=False, debug=False, dynamic_dma_scratch_size=16384
    )
    build_program(nc, bass, mybir, tile)
    nc.compile()
    _NC_CACHE["nc"] = nc
    return nc


def run_on_hw(in_maps, trace=False, **kwargs):
    _ensure_paths()
    from concourse.bass_utils import run_bass_kernel_spmd

    nc = _get_nc()
    return run_bass_kernel_spmd(
        nc, in_maps, core_ids=list(range(N_CORES)), trace=trace, **kwargs
    )


def kernel(x_coarse, keep_idx, E_fine=None, **_unused):
    in_maps = host_inputs(x_coarse, keep_idx)
    res = run_on_hw(in_maps)
    out = np.concatenate([res.results[m]["y"] for m in range(N_CORES)], axis=0)
    return np.ascontiguousarray(out.astype(np.float32, copy=False))


# revision 20
# speedup vs baseline: 1.1310x; 1.0514x over previous
"""MeshUnpool Trainium2 kernel (v3).

For every fine edge slot s in [0, 16384):
  - if s is a kept slot (s == keep_idx[j] for some j): out[s] = x_coarse[j]
  - else: out[s] = x_coarse[argmin_j |keep_idx[j] - s|]  (first-min tie-break)

Algorithm (per core; front-end replicated, each core extracts its slice):
  1. slot table via matmul scatter in chunk-major layout (PE operands
     contiguous). Payload u = jp+1 rides a 4x tensor_scalar; payload v = c
     rides a host-constant c-ramp multiply (2x). One-hot compares split
     between vector and gpsimd engines.
  2. dual-f32-key prefix-max / suffix-min scans give nearest kept slot
     left/right with (u, v) riding along; cross-partition carry via PE
     transpose + exclusive scan.
  3. integer decode + first-min tie-break -> src row per slot.
  4. one sel-matmul extracts this core's 2048 indices; 4 multi-offset
     indirect DMAs (512 rows each) gather x_coarse rows; 4 strided DMAs
     write the slice of y.

Sharding: rows of the fine-edge dim, 2048 per core; x_coarse/keep replicated.
"""

import os
import sys

import numpy as np

E_FINE = 16384
E_COARSE = 8192
C = 512
N_CORES = 8
SLICE = E_FINE // N_CORES  # 2048
P = 128
KC = E_COARSE // P  # 64 keep chunks (j = 128*c + jp)
NPC = 4  # build pieces
PCK = KC // NPC  # chunks per piece
NG = 4  # gather/write groups
GB = 4  # blocks of 128 rows per group

R_SENT = 8388608.0  # 2^23 sentinel for the suffix-min scans

A_ALL_ON_GPSIMD = False  # risk flag: one-hot A build on Pool engine
MULTI_COL_INDIRECT = False  # risk flag: [128, 4] offset APs per indirect DMA

_NC_CACHE = {}
_DUMP = None  # debug hook: _DUMP(name, ap) dumps an AP to a DRAM tensor


def _dump(name, ap):
    if _DUMP is not None:
        _DUMP(name, ap)


def _ensure_paths():
    for p in ("/opt/trn_rl_repo", "/root/.axon_site/_ro/trn_rl_repo"):
        if os.path.isdir(p) and p not in sys.path:
            sys.path.append(p)


def build_program(nc, bass, mybir, tile):
    f32 = mybir.dt.float32
    i32 = mybir.dt.int32
    bf16 = mybir.dt.bfloat16
    Alu = mybir.AluOpType

    xc = nc.dram_tensor("xc", [E_COARSE, C], f32, kind="ExternalInput")
    # iota[p, t] = t
    bpk = nc.dram_tensor("bpk", [P, P], bf16, kind="ExternalInput")
    # hbf/lbf[jp, cc, t] = keep_idx[128*(16k+cc)+jp] >>7 / &127 (repeated
    # along t); crp[jp, cc, t] = global chunk index (c-ramp payload)
    hbfs = [
        nc.dram_tensor(f"hbf{k}", [P, PCK, P], bf16, kind="ExternalInput")
        for k in range(NPC)
    ]
    lbfs = [
        nc.dram_tensor(f"lbf{k}", [P, PCK, P], bf16, kind="ExternalInput")
        for k in range(NPC)
    ]
    crps = [
        nc.dram_tensor(f"crp{k}", [P, PCK, P], bf16, kind="ExternalInput")
        for k in range(NPC)
    ]
    # f32 pack: pos128m1 | pos64 | jp1 | sel | ident
    fpk = nc.dram_tensor("fpk", [P, 401], f32, kind="ExternalInput")
    # i32 pack: pos
    ipk = nc.dram_tensor("ipk", [P, P], i32, kind="ExternalInput")
    y = nc.dram_tensor("y", [SLICE, C], f32, kind="ExternalOutput")

    with tile.TileContext(nc) as tc:
        with (
            tc.tile_pool(name="sb", bufs=1) as sb,
            tc.tile_pool(name="ps", bufs=1, space="PSUM") as ps,
        ):
            # ---- input loads (spread over both HWDGE engines)
            bpk_t = sb.tile([P, P], bf16)
            nc.sync.dma_start(bpk_t[:], bpk[:])
            fpk_t = sb.tile([P, 401], f32)
            nc.sync.dma_start(fpk_t[:], fpk[:])
            pos_i = sb.tile([P, P], i32)
            nc.sync.dma_start(pos_i[:], ipk[:])

            iota = bpk_t[:, 0:P]
            pos128m1 = fpk_t[:, 0:128]
            pos64 = fpk_t[:, 128:256]
            jp1 = fpk_t[:, 256:257]
            sel = fpk_t[:, 257:273]
            ident = fpk_t[:, 273:401]

            tps = ps.tile([P, 2 * P], f32)
            a_eng = nc.gpsimd if A_ALL_ON_GPSIMD else nc.vector

            # ---- one-hot builds + matmul scatter, piecewise over chunks
            for k in range(NPC):
                hbf_t = sb.tile([P, PCK, P], bf16, tag=f"hbf{k}")
                nc.scalar.dma_start(hbf_t[:], hbfs[k][:])
                lbf_t = sb.tile([P, PCK, P], bf16, tag=f"lbf{k}")
                nc.scalar.dma_start(lbf_t[:], lbfs[k][:])
                crp_t = sb.tile([P, PCK, P], bf16, tag=f"crp{k}")
                nc.scalar.dma_start(crp_t[:], crps[k][:])
                # aA[jp, cc, t] = (hb[jp, c] == t)
                aA = sb.tile([P, PCK, P], bf16, tag=f"aA{k}")
                a_eng.tensor_tensor(
                    aA[:],
                    hbf_t[:],
                    iota.unsqueeze(1).to_broadcast([P, PCK, P]),
                    Alu.is_equal,
                )
                # cT[jp, cc, t] = (lb[jp, c] == t)
                cT = sb.tile([P, PCK, P], bf16, tag=f"cT{k}")
                nc.vector.tensor_tensor(
                    cT[:],
                    lbf_t[:],
                    iota.unsqueeze(1).to_broadcast([P, PCK, P]),
                    Alu.is_equal,
                )
                # pay_u = (jp+1) * cT ; pay_v = c * cT
                pay = sb.tile([P, PCK, 2 * P], bf16, tag=f"pay{k}")
                nc.vector.tensor_scalar(
                    pay[:, :, 0:P], cT[:], jp1, None, Alu.mult
                )
                nc.vector.tensor_tensor(
                    pay[:, :, P : 2 * P], cT[:], crp_t[:], Alu.mult
                )
                for cc in range(PCK):
                    c = k * PCK + cc
                    nc.tensor.matmul(
                        tps[:],
                        aA[:, cc, :],
                        pay[:, cc, :],
                        start=(c == 0),
                        stop=(c == KC - 1),
                    )

            # ---- keys: key1 = kept*(128*pos + jp), key2 = kept*(64*pos + c)
            mk = sb.tile([P, P], f32)
            nc.vector.tensor_scalar(mk[:], tps[:, 0:P], 0.0, None, Alu.is_gt)
            k1r = sb.tile([P, P], f32)
            nc.vector.tensor_tensor(k1r[:], tps[:, 0:P], pos128m1, Alu.add)
            key1 = sb.tile([P, P], f32)
            nc.vector.tensor_tensor(key1[:], k1r[:], mk[:], Alu.mult)
            k2r = sb.tile([P, P], f32)
            nc.vector.tensor_tensor(k2r[:], tps[:, P : 2 * P], pos64, Alu.add)
            key2 = sb.tile([P, P], f32)
            nc.vector.tensor_tensor(key2[:], k2r[:], mk[:], Alu.mult)
            msk = sb.tile([P, P], f32)
            nc.vector.tensor_scalar(msk[:], key1[:], 0.0, None, Alu.is_equal)
            r1 = sb.tile([P, P], f32)
            nc.vector.scalar_tensor_tensor(
                r1[:], msk[:], R_SENT, key1[:], Alu.mult, Alu.add
            )
            r2 = sb.tile([P, P], f32)
            nc.vector.scalar_tensor_tensor(
                r2[:], msk[:], R_SENT, key2[:], Alu.mult, Alu.add
            )
            _dump("d_key1", key1[:])
            _dump("d_key2", key2[:])

            # ---- per-partition scans; l1s|l2s|r1s|r2s packed in one tile
            sc = sb.tile([P, 4 * P], f32)
            nc.vector.tensor_tensor_scan(
                sc[:, 0:P], key1[:], key1[:], 0.0, Alu.max, Alu.max
            )
            nc.vector.tensor_tensor_scan(
                sc[:, P : 2 * P], key2[:], key2[:], 0.0, Alu.max, Alu.max
            )
            nc.vector.tensor_tensor_scan(
                sc[:, 3 * P - 1 : 2 * P - 1 : -1],
                r1[:, P - 1 :: -1],
                r1[:, P - 1 :: -1],
                R_SENT,
                Alu.min,
                Alu.min,
            )
            nc.vector.tensor_tensor_scan(
                sc[:, 4 * P - 1 : 3 * P - 1 : -1],
                r2[:, P - 1 :: -1],
                r2[:, P - 1 :: -1],
                R_SENT,
                Alu.min,
                Alu.min,
            )

            # ---- cross-partition carries: transpose totals, exclusive scan
            totL_ps = ps.tile([2, P], f32)
            nc.tensor.transpose(totL_ps[:], sc[:, P - 1 : 2 * P : P], ident)
            totR_ps = ps.tile([2, P], f32)
            nc.tensor.transpose(totR_ps[:], sc[:, 2 * P : 3 * P + 1 : P], ident)
            totL_T = sb.tile([2, P], f32)
            nc.vector.tensor_copy(totL_T[:], totL_ps[:])
            totR_T = sb.tile([2, P], f32)
            nc.vector.tensor_copy(totR_T[:], totR_ps[:])
            exL = sb.tile([2, P], f32)
            nc.vector.memset(exL[:, 0:1], 0.0)
            nc.vector.tensor_tensor_scan(
                exL[:, 1:P],
                totL_T[:, 0 : P - 1],
                totL_T[:, 0 : P - 1],
                0.0,
                Alu.max,
                Alu.max,
            )
            exR = sb.tile([2, P], f32)
            nc.vector.memset(exR[:, P - 1 : P], R_SENT)
            nc.vector.tensor_tensor_scan(
                exR[:, P - 2 :: -1],
                totR_T[:, P - 1 : 0 : -1],
                totR_T[:, P - 1 : 0 : -1],
                R_SENT,
                Alu.min,
                Alu.min,
            )
            carL_ps = ps.tile([P, 2], f32)
            nc.tensor.transpose(carL_ps[:], exL[:], ident[0:2, 0:2])
            carR_ps = ps.tile([P, 2], f32)
            nc.tensor.transpose(carR_ps[:], exR[:], ident[0:2, 0:2])
            carL = sb.tile([P, 2], f32)
            nc.vector.tensor_copy(carL[:], carL_ps[:])
            carR = sb.tile([P, 2], f32)
            nc.vector.tensor_copy(carR[:], carR_ps[:])

            # ---- combine carries (f32), then integer decode
            nc.vector.tensor_scalar_max(sc[:, 0:P], sc[:, 0:P], carL[:, 0:1])
            nc.vector.tensor_scalar_max(
                sc[:, P : 2 * P], sc[:, P : 2 * P], carL[:, 1:2]
            )
            nc.vector.tensor_scalar_min(
                sc[:, 2 * P : 3 * P], sc[:, 2 * P : 3 * P], carR[:, 0:1]
            )
            nc.vector.tensor_scalar_min(
                sc[:, 3 * P : 4 * P], sc[:, 3 * P : 4 * P], carR[:, 1:2]
            )
            l1m = sb.tile([P, P], i32)
            nc.vector.tensor_copy(l1m[:], sc[:, 0:P])
            l2m = sb.tile([P, P], i32)
            nc.vector.tensor_copy(l2m[:], sc[:, P : 2 * P])
            r1m = sb.tile([P, P], i32)
            nc.vector.tensor_copy(r1m[:], sc[:, 2 * P : 3 * P])
            r2m = sb.tile([P, P], i32)
            nc.vector.tensor_copy(r2m[:], sc[:, 3 * P : 4 * P])
            _dump("d_l1s", l1m[:])
            _dump("d_r1s", r1m[:])

            slot_l = sb.tile([P, P], i32)
            nc.vector.tensor_scalar(slot_l[:], l1m[:], 7, None, Alu.arith_shift_right)
            slot_r = sb.tile([P, P], i32)
            nc.vector.tensor_scalar(slot_r[:], r1m[:], 7, None, Alu.arith_shift_right)
            # j = 128*c + jp  (u = jp+1 was the payload, key stores jp; v = c)
            jcl = sb.tile([P, P], i32)
            nc.vector.tensor_scalar(
                jcl[:], l2m[:], 63, 7, Alu.bitwise_and, Alu.arith_shift_left
            )
            jpl = sb.tile([P, P], i32)
            nc.vector.tensor_scalar(jpl[:], l1m[:], 127, None, Alu.bitwise_and)
            jl = sb.tile([P, P], i32)
            nc.vector.tensor_tensor(jl[:], jcl[:], jpl[:], Alu.bitwise_or)
            jcr = sb.tile([P, P], i32)
            nc.vector.tensor_scalar(
                jcr[:], r2m[:], 63, 7, Alu.bitwise_and, Alu.arith_shift_left
            )
            jpr = sb.tile([P, P], i32)
            nc.vector.tensor_scalar(jpr[:], r1m[:], 127, None, Alu.bitwise_and)
            jr = sb.tile([P, P], i32)
            nc.vector.tensor_tensor(jr[:], jcr[:], jpr[:], Alu.bitwise_or)

            dl = sb.tile([P, P], i32)
            nc.vector.tensor_tensor(dl[:], pos_i[:], slot_l[:], Alu.subtract)
            drr = sb.tile([P, P], i32)
            nc.vector.tensor_tensor(drr[:], slot_r[:], pos_i[:], Alu.subtract)
            ml = sb.tile([P, P], i32)
            nc.vector.tensor_tensor(ml[:], dl[:], drr[:], Alu.is_lt)
            mr = sb.tile([P, P], i32)
            nc.vector.tensor_tensor(mr[:], drr[:], dl[:], Alu.is_lt)
            src = sb.tile([P, P], i32)
            nc.vector.tensor_tensor(src[:], jl[:], jr[:], Alu.min)
            nc.vector.copy_predicated(src[:], mr[:], jr[:])
            nc.vector.copy_predicated(src[:], ml[:], jl[:])
            _dump("d_src", src[:])
            srcf = sb.tile([P, P], f32)
            nc.vector.tensor_copy(srcf[:], src[:])

            # ---- extract this core's 2048 indices: g_i[p, b] = src row for
            # slot 2048m + 128b + p
            g_ps = ps.tile([P, 16], f32)
            nc.tensor.matmul(g_ps[:], srcf[:], sel, start=True, stop=True)
            g_i = sb.tile([P, 16], i32)
            nc.vector.tensor_copy(g_i[:], g_ps[:])
            _dump("d_gi", g_i[:])

            # ---- gather + write: per-block indirect gathers (v1 pattern)
            with tc.tile_pool(name="gp", bufs=16) as gp:
                for b in range(16):
                    gt = gp.tile([P, C], f32, tag="g")
                    nc.gpsimd.indirect_dma_start(
                        out=gt[:],
                        out_offset=None,
                        in_=xc[:],
                        in_offset=bass.IndirectOffsetOnAxis(
                            ap=g_i[:, b : b + 1], axis=0
                        ),
                    )
                    nc.sync.dma_start(y[b * P : (b + 1) * P, :], gt[:])

    return {"y": y}


def host_inputs(x_coarse, keep_idx):
    import ml_dtypes

    bf = ml_dtypes.bfloat16
    x_coarse = np.ascontiguousarray(np.asarray(x_coarse), dtype=np.float32)
    ki = np.ascontiguousarray(np.asarray(keep_idx), dtype=np.int32).reshape(-1)
    # j = 128*c + jp: keep_w[jp, c] = keep_idx[j]
    keep_w = ki.reshape(KC, P).T
    hbf_full = np.broadcast_to(
        (keep_w >> 7).astype(bf)[:, :, None], (P, KC, P)
    )
    lbf_full = np.broadcast_to(
        (keep_w & 127).astype(bf)[:, :, None], (P, KC, P)
    )

    t = np.arange(P)
    iota_a = np.tile(t[None, :], (P, 1)).astype(bf)
    crp_full = np.tile(np.arange(KC)[None, :, None], (P, 1, P)).astype(bf)

    pos = (16384 + 128 * t[:, None] + t[None, :]).astype(np.int64)
    pos128m1 = (128 * pos - 1).astype(np.float32)
    pos64 = (64 * pos).astype(np.float32)
    jp1 = (t[:, None] + 1).astype(np.float32)
    ident_a = np.eye(P, dtype=np.float32)

    base = {
        "xc": x_coarse,
        "bpk": np.ascontiguousarray(iota_a),
        "ipk": np.ascontiguousarray(pos.astype(np.int32)),
    }
    for k in range(NPC):
        cs = slice(k * PCK, (k + 1) * PCK)
        base[f"hbf{k}"] = np.ascontiguousarray(hbf_full[:, cs, :])
        base[f"lbf{k}"] = np.ascontiguousarray(lbf_full[:, cs, :])
        base[f"crp{k}"] = np.ascontiguousarray(crp_full[:, cs, :])

    in_maps = []
    for m in range(N_CORES):
        sel_a = np.zeros((P, 16), dtype=np.float32)
        sel_a[16 * m + np.arange(16), np.arange(16)] = 1.0
        fpk_a = np.concatenate([pos128m1, pos64, jp1, sel_a, ident_a], axis=1)
        in_maps.append(dict(base, fpk=np.ascontiguousarray(fpk_a)))
    return in_maps


def _get_nc():
    if "nc" in _NC_CACHE:
        return _NC_CACHE["nc"]
    _ensure_paths()
    from concourse import bass, mybir
    import concourse.bacc as bacc
    import concourse.tile as tile

    nc = bacc.Bacc(
        "TRN2", target_bir_lowering=False, debug=False, dynamic_dma_scratch_size=16384
    )
    build_program(nc, bass, mybir, tile)
    nc.compile()
    _NC_CACHE["nc"] = nc
    return nc


def run_on_hw(in_maps, trace=False, **kwargs):
    _ensure_paths()
    from concourse.bass_utils import run_bass_kernel_spmd

    nc = _get_nc()
    return run_bass_kernel_spmd(
        nc, in_maps, core_ids=list(range(N_CORES)), trace=trace, **kwargs
    )


def kernel(x_coarse, keep_idx, E_fine=None, **_unused):
    in_maps = host_inputs(x_coarse, keep_idx)
    res = run_on_hw(in_maps)
    out = np.concatenate([res.results[m]["y"] for m in range(N_CORES)], axis=0)
    return np.ascontiguousarray(out.astype(np.float32, copy=False))
